# revision 1
# baseline (speedup 1.0000x reference)
"""CrossAttentionLayer Trainium2 kernel, 8-way sharded.

Sharding: core c -> batch b = c//4, head-group/token-slice r = c%4.
- q/k/v projections column-sharded over heads (4 heads = 512 dims per core)
- attention computed per head entirely in feature-major layout (no transposes)
- out-projection row-sharded (Megatron): partial [2048, 2048] per core in 4
  token-block chunks, each followed by a bf16 ReduceScatter(add) over the
  4 cores of the batch; the gate GEMM runs after as PE filler so the
  collectives are fully hidden
- sigmoid gate + residual + LayerNorm on the core's 512-token slice

Everything per-core varying is prepared host-side (transposes, slices,
broadcast biases), so the single SPMD program is identical on all cores.

GEMMs run in bf16 with fp32 PSUM accumulation. The softmax path keeps full
precision by operating on em = exp(s)-1 (values ~1e-3, full relative
precision in bf16); the "1" part of every probability is carried exactly
through per-head v column sums and the constant 2048 in the denominator.
The attention path's absolute contribution to the output is ~2e-4 of the
residual, so bf16 GEMM noise lands ~1e-5 relative on the final output.
"""

import os

import numpy as np

import concourse.bacc as bacc
import concourse.mybir as mybir
import concourse.tile as tile
from concourse.bass_utils import run_bass_kernel_spmd

H = 2048          # hidden
S = 2048          # sequence
B = 2             # batch
HD = 128          # head dim
P = 128           # partitions
QD = 512          # per-core qkv dims (4 heads)
TS = 512          # per-core token slice
KT = H // P       # 16 contraction tiles
ST = S // P       # 16 token tiles
SCALE = HD ** -0.5
EPS = 1e-5

F32 = mybir.dt.float32
BF16 = mybir.dt.bfloat16
FA = mybir.ActivationFunctionType
OP = mybir.AluOpType

TRACE = False          # test.py sets True to capture an NTFF profile
LAST_RESULT = None     # BassKernelResults from the most recent run

_CACHE = {}


def _build():
    from contextlib import ExitStack

    nc = bacc.Bacc("TRN2", target_bir_lowering=False, debug=False, num_devices=8)

    hidT = nc.dram_tensor("hidT", [H, S], BF16, kind="ExternalInput")
    crossT = nc.dram_tensor("crossT", [H, S], BF16, kind="ExternalInput")
    hsliT = nc.dram_tensor("hsliT", [H, TS], BF16, kind="ExternalInput")
    hsli = nc.dram_tensor("hsli", [TS, H], F32, kind="ExternalInput")
    wq = nc.dram_tensor("wq", [H, QD], BF16, kind="ExternalInput")
    wk = nc.dram_tensor("wk", [H, QD], BF16, kind="ExternalInput")
    wv = nc.dram_tensor("wv", [H, QD], BF16, kind="ExternalInput")
    wo = nc.dram_tensor("wo", [QD, H], BF16, kind="ExternalInput")
    wg = nc.dram_tensor("wg", [H, H], BF16, kind="ExternalInput")
    bq = nc.dram_tensor("bq", [4, P, 1], F32, kind="ExternalInput")
    bk = nc.dram_tensor("bk", [4, P, 1], F32, kind="ExternalInput")
    bvb = nc.dram_tensor("bvb", [P, QD], F32, kind="ExternalInput")
    bob = nc.dram_tensor("bob", [P, H], F32, kind="ExternalInput")
    bgb = nc.dram_tensor("bgb", [P, H], F32, kind="ExternalInput")
    gmb = nc.dram_tensor("gmb", [P, H], F32, kind="ExternalInput")
    btb = nc.dram_tensor("btb", [P, H], F32, kind="ExternalInput")
    y = nc.dram_tensor("y", [TS, H], F32, kind="ExternalOutput")

    groups = [[0, 1, 2, 3], [4, 5, 6, 7]]

    with tile.TileContext(nc) as tc, ExitStack() as top:
        const = top.enter_context(tc.tile_pool(name="const", bufs=1))
        ones_sq = const.tile([P, P], BF16, name="ones_sq")
        nc.gpsimd.memset(ones_sq[:], 1.0)
        ones_col = const.tile([P, 1], BF16, name="ones_col")
        nc.gpsimd.memset(ones_col[:], 1.0)
        eps_t = const.tile([P, 1], F32, name="eps_t")
        nc.gpsimd.memset(eps_t[:], EPS)
        bq_t = [const.tile([P, 1], F32, name=f"bq{m}") for m in range(4)]
        bk_t = [const.tile([P, 1], F32, name=f"bk{m}") for m in range(4)]
        for m in range(4):
            nc.sync.dma_start(bq_t[m][:], bq[m])
            nc.sync.dma_start(bk_t[m][:], bk[m])
        bvb_sb = const.tile([P, QD], F32, name="bvb_sb")
        nc.sync.dma_start(bvb_sb[:], bvb[:])
        bo_sb = const.tile([P, H], F32, name="bo_sb")
        nc.sync.dma_start(bo_sb[:], bob[:])
        bg_sb = const.tile([P, H], F32, name="bg_sb")
        nc.sync.dma_start(bg_sb[:], bgb[:])
        gm_sb = const.tile([P, H], F32, name="gm_sb")
        nc.sync.dma_start(gm_sb[:], gmb[:])
        bt_sb = const.tile([P, H], F32, name="bt_sb")
        nc.sync.dma_start(bt_sb[:], btb[:])

        cc = top.enter_context(tc.tile_pool(name="cc", bufs=1, space="DRAM"))
        cc_in = cc.tile([S, H], BF16, name="ccin")
        cc_out = cc.tile([TS, H], BF16, name="ccout")

        hidT_r = hidT.rearrange("(t p) s -> t p s", p=P)
        crossT_r = crossT.rearrange("(t p) s -> t p s", p=P)
        wq_r = wq.rearrange("(t p) d -> t p d", p=P)
        wk_r = wk.rearrange("(t p) d -> t p d", p=P)
        wv_r = wv.rearrange("(t p) d -> t p d", p=P)
        wo_r = wo.rearrange("(t p) d -> t p d", p=P)
        wg_r = wg.rearrange("(t p) d -> t p d", p=P)
        hsliT_r = hsliT.rearrange("(t p) s -> t p s", p=P)

        with ExitStack() as ab:
            # ---- persistent activations for phases A+B+C ----
            qkv = ab.enter_context(tc.tile_pool(name="qkv", bufs=1))
            q_sb = [qkv.tile([P, S], BF16, name=f"q{m}") for m in range(4)]
            k_sb = [qkv.tile([P, S], BF16, name=f"k{m}") for m in range(4)]
            v_sb = [qkv.tile([P, QD], BF16, name=f"v{t}") for t in range(ST)]
            attnT = [qkv.tile([P, S], BF16, name=f"at{m}") for m in range(4)]

            # ---- phase A: q projection ----
            with ExitStack() as ph:
                wp = ph.enter_context(tc.tile_pool(name="wp", bufs=1))
                xp = ph.enter_context(tc.tile_pool(name="xp", bufs=6))
                psA = ph.enter_context(tc.tile_pool(name="psA", bufs=8, space="PSUM"))
                wq_sb = [wp.tile([P, QD], BF16, name=f"wq{k}") for k in range(KT)]
                for k in range(KT):
                    nc.sync.dma_start(wq_sb[k][:], wq_r[k])
                for c in range(4):
                    ps_q = [psA.tile([P, 512], F32, name="psq") for _ in range(4)]
                    for k in range(KT):
                        x = xp.tile([P, 512], BF16, name="x")
                        nc.sync.dma_start(x[:], hidT_r[k, :, c * 512:(c + 1) * 512])
                        for m in range(4):
                            nc.tensor.matmul(
                                ps_q[m][:], wq_sb[k][:, m * P:(m + 1) * P], x[:],
                                start=(k == 0), stop=(k == KT - 1))
                    for m in range(4):
                        nc.scalar.activation(
                            q_sb[m][:, c * 512:(c + 1) * 512], ps_q[m][:],
                            FA.Identity, bias=bq_t[m][:])

            # ---- phase A: k and v projections (one crossT pass) ----
            with ExitStack() as ph:
                wp = ph.enter_context(tc.tile_pool(name="wp2", bufs=1))
                xp = ph.enter_context(tc.tile_pool(name="xp2", bufs=6))
                psA = ph.enter_context(tc.tile_pool(name="psA2", bufs=4, space="PSUM"))
                wk_sb = [wp.tile([P, QD], BF16, name=f"wk{k}") for k in range(KT)]
                wv_sb = [wp.tile([P, QD], BF16, name=f"wv{k}") for k in range(KT)]
                for k in range(KT):
                    nc.sync.dma_start(wk_sb[k][:], wk_r[k])
                    nc.sync.dma_start(wv_sb[k][:], wv_r[k])
                for c in range(4):
                    ps_k = [psA.tile([P, 512], F32, name="psk") for _ in range(4)]
                    ps_v = [psA.tile([P, 512], F32, name="psv") for _ in range(4)]
                    for k in range(KT):
                        x = xp.tile([P, 512], BF16, name="x2")
                        nc.sync.dma_start(x[:], crossT_r[k, :, c * 512:(c + 1) * 512])
                        for m in range(4):
                            nc.tensor.matmul(
                                ps_k[m][:], wk_sb[k][:, m * P:(m + 1) * P], x[:],
                                start=(k == 0), stop=(k == KT - 1))
                        for t in range(4):
                            nc.tensor.matmul(
                                ps_v[t][:], x[:, t * P:(t + 1) * P], wv_sb[k][:],
                                start=(k == 0), stop=(k == KT - 1))
                    for m in range(4):
                        nc.scalar.activation(
                            k_sb[m][:, c * 512:(c + 1) * 512], ps_k[m][:],
                            FA.Identity, bias=bk_t[m][:])
                    for t in range(4):
                        nc.vector.tensor_add(
                            v_sb[c * 4 + t][:], ps_v[t][:], bvb_sb[:])

            # ---- phase B: attention per head ----
            with ExitStack() as ph:
                psS = ph.enter_context(tc.tile_pool(name="psS", bufs=4, space="PSUM"))
                psAcc = ph.enter_context(tc.tile_pool(name="psAcc", bufs=1, space="PSUM"))
                psVs = ph.enter_context(tc.tile_pool(name="psVs", bufs=1, space="PSUM"))
                exp_p = ph.enter_context(tc.tile_pool(name="exp", bufs=8))
                em_p = ph.enter_context(tc.tile_pool(name="em", bufs=10))
                tmp_p = ph.enter_context(tc.tile_pool(name="tmpB", bufs=4))
                vs_p = ph.enter_context(tc.tile_pool(name="vs", bufs=1))
                vs_sb = [vs_p.tile([P, 1], F32, name=f"vs{h}") for h in range(4)]
                for h in range(4):
                    # per-head column sums of v (the "1" part of exp = 1 + em)
                    ps_vs = psVs.tile([P, 1], F32, name="psvs")
                    for t in range(ST):
                        nc.tensor.matmul(
                            ps_vs[:], v_sb[t][:, h * P:(h + 1) * P], ones_col[:],
                            start=(t == 0), stop=(t == ST - 1))
                    nc.scalar.activation(vs_sb[h][:], ps_vs[:], FA.Identity)
                    for c in range(4):
                        ps_at = psAcc.tile([P, 512], F32, name="psat")
                        ps_sum = psAcc.tile([P, 512], F32, name="pssum")
                        for t in range(ST):
                            ps_sc = psS.tile([P, 512], F32, name="pssc")
                            nc.tensor.matmul(
                                ps_sc[:], k_sb[h][:, t * P:(t + 1) * P],
                                q_sb[h][:, c * 512:(c + 1) * 512],
                                start=True, stop=True)
                            ex = exp_p.tile([P, 512], F32, name="ex")
                            nc.scalar.activation(ex[:], ps_sc[:], FA.Exp, scale=SCALE)
                            em = em_p.tile([P, 512], BF16, name="em")
                            nc.vector.tensor_scalar_add(em[:], ex[:], -1.0)
                            nc.tensor.matmul(
                                ps_at[:], v_sb[t][:, h * P:(h + 1) * P], em[:],
                                start=(t == 0), stop=(t == ST - 1))
                            nc.tensor.matmul(
                                ps_sum[:], ones_sq[:], em[:],
                                start=(t == 0), stop=(t == ST - 1))
                        den = tmp_p.tile([P, 512], F32, name="den")
                        nc.vector.tensor_scalar_add(den[:], ps_sum[:], float(S))
                        rec = tmp_p.tile([P, 512], F32, name="rec")
                        nc.vector.reciprocal(rec[:], den[:])
                        num = tmp_p.tile([P, 512], F32, name="num")
                        nc.vector.tensor_scalar_add(num[:], ps_at[:], vs_sb[h][:])
                        nc.vector.tensor_mul(
                            attnT[h][:, c * 512:(c + 1) * 512], num[:], rec[:])

            # ---- phase C: out-projection partial, 4 token-block RS chunks ----
            with ExitStack() as ph:
                wop = ph.enter_context(tc.tile_pool(name="wop", bufs=1))
                psC = ph.enter_context(tc.tile_pool(name="psC", bufs=4, space="PSUM"))
                stg = ph.enter_context(tc.tile_pool(name="stg", bufs=4))
                wo_sb = [wop.tile([P, H], BF16, name=f"wo{k}") for k in range(4)]
                for k in range(4):
                    nc.sync.dma_start(wo_sb[k][:], wo_r[k])
                for t in range(ST):
                    for n in range(4):
                        ps_o = psC.tile([P, 512], F32, name="pso")
                        for k in range(4):
                            nc.tensor.matmul(
                                ps_o[:], attnT[k][:, t * P:(t + 1) * P],
                                wo_sb[k][:, n * 512:(n + 1) * 512],
                                start=(k == 0), stop=(k == 3))
                        st = stg.tile([P, 512], BF16, name="st")
                        nc.scalar.copy(st[:], ps_o[:])
                        nc.sync.dma_start(
                            cc_in[t * P:(t + 1) * P,
                                  n * 512:(n + 1) * 512], st[:])
                nc.gpsimd.collective_compute(
                    "ReduceScatter", OP.add, replica_groups=groups,
                    ins=[cc_in[:].opt()], outs=[cc_out[:].opt()])

        # ---- phase D: gate GEMM (PE filler under the collectives) ----
        with ExitStack() as ph:
            g_pool = ph.enter_context(tc.tile_pool(name="gp", bufs=1))
            hsl_p = ph.enter_context(tc.tile_pool(name="hsl", bufs=1))
            wgp = ph.enter_context(tc.tile_pool(name="wgp", bufs=6))
            psG = ph.enter_context(tc.tile_pool(name="psG", bufs=4, space="PSUM"))
            fin = ph.enter_context(tc.tile_pool(name="fin", bufs=2))
            sml = ph.enter_context(tc.tile_pool(name="sml", bufs=4))

            hsl_sb = [hsl_p.tile([P, 512], BF16, name=f"hs{k}") for k in range(KT)]
            for k in range(KT):
                nc.sync.dma_start(hsl_sb[k][:], hsliT_r[k])
            g_sb = [g_pool.tile([P, H], F32, name=f"g{m}") for m in range(4)]
            for n in range(4):
                ps_g = [psG.tile([P, 512], F32, name="psg") for _ in range(4)]
                for k in range(KT):
                    wgt = wgp.tile([P, 512], BF16, name="wgt")
                    nc.sync.dma_start(wgt[:], wg_r[k, :, n * 512:(n + 1) * 512])
                    for m in range(4):
                        nc.tensor.matmul(
                            ps_g[m][:], hsl_sb[k][:, m * P:(m + 1) * P], wgt[:],
                            start=(k == 0), stop=(k == KT - 1))
                for m in range(4):
                    t = fin.tile([P, 512], F32, name="gpre")
                    nc.vector.tensor_add(
                        t[:], ps_g[m][:], bg_sb[:, n * 512:(n + 1) * 512])
                    nc.scalar.activation(
                        g_sb[m][:, n * 512:(n + 1) * 512], t[:], FA.Sigmoid)

            # ---- phase E: combine + LayerNorm per token tile ----
            # input DMAs ride the gpsimd queue: it is ordered after the
            # collectives, so the sync queue never head-of-line blocks on CC
            for m in range(4):
                ob = fin.tile([P, H], BF16, name="ob")
                nc.gpsimd.dma_start(ob[:], cc_out[m * P:(m + 1) * P, :])
                x = fin.tile([P, H], F32, name="xres")
                nc.gpsimd.dma_start(x[:], hsli[m * P:(m + 1) * P, :])
                o = fin.tile([P, H], F32, name="o")
                nc.vector.tensor_add(o[:], ob[:], bo_sb[:])
                nc.vector.tensor_mul(o[:], o[:], g_sb[m][:])
                nc.vector.tensor_add(o[:], o[:], x[:])
                ssum = sml.tile([P, 1], F32, name="ssum")
                nc.vector.reduce_sum(ssum[:], o[:], axis=mybir.AxisListType.X)
                nmean = sml.tile([P, 1], F32, name="nmean")
                nc.scalar.mul(nmean[:], ssum[:], -1.0 / H)
                nc.vector.tensor_scalar_add(o[:], o[:], nmean[:])
                sq = fin.tile([P, H], F32, name="sq")
                ssq = sml.tile([P, 1], F32, name="ssq")
                nc.vector.tensor_mul(sq[:], o[:], o[:])
                nc.vector.reduce_sum(ssq[:], sq[:], axis=mybir.AxisListType.X)
                sd = sml.tile([P, 1], F32, name="sd")
                nc.scalar.activation(sd[:], ssq[:], FA.Sqrt, bias=eps_t[:], scale=1.0 / H)
                rstd = sml.tile([P, 1], F32, name="rstd")
                nc.vector.reciprocal(rstd[:], sd[:])
                nc.vector.tensor_scalar_mul(o[:], o[:], rstd[:])
                nc.vector.tensor_mul(o[:], o[:], gm_sb[:])
                nc.vector.tensor_add(o[:], o[:], bt_sb[:])
                nc.sync.dma_start(y[m * P:(m + 1) * P, :], o[:])

    nc.compile()
    return nc


def kernel(**inputs):
    global LAST_RESULT
    import ml_dtypes

    if "nc" not in _CACHE:
        _CACHE["nc"] = _build()
    nc = _CACHE["nc"]

    bf16 = ml_dtypes.bfloat16
    hs = np.asarray(inputs["hidden_states"], dtype=np.float32)
    cs = np.asarray(inputs["cross_states"], dtype=np.float32)
    Wq = np.asarray(inputs["Wq"], dtype=np.float32)
    Wk = np.asarray(inputs["Wk"], dtype=np.float32)
    Wv = np.asarray(inputs["Wv"], dtype=np.float32)
    Wo = np.asarray(inputs["Wo"], dtype=np.float32)
    Wg = np.asarray(inputs["Wg"], dtype=np.float32).astype(bf16)
    bq = np.asarray(inputs["bq"], dtype=np.float32)
    bk = np.asarray(inputs["bk"], dtype=np.float32)
    bv = np.asarray(inputs["bv"], dtype=np.float32)
    bo = np.asarray(inputs["bo"], dtype=np.float32)
    bg = np.asarray(inputs["bg"], dtype=np.float32)
    gm = np.asarray(inputs["ln_gamma"], dtype=np.float32)
    bt = np.asarray(inputs["ln_beta"], dtype=np.float32)

    bob = np.ascontiguousarray(np.broadcast_to(bo, (P, H)))
    bgb = np.ascontiguousarray(np.broadcast_to(bg, (P, H)))
    gmb = np.ascontiguousarray(np.broadcast_to(gm, (P, H)))
    btb = np.ascontiguousarray(np.broadcast_to(bt, (P, H)))

    in_maps = []
    for c in range(8):
        b, r = divmod(c, 4)
        sl = slice(r * QD, (r + 1) * QD)
        tsl = slice(r * TS, (r + 1) * TS)
        hT = np.ascontiguousarray(hs[b].T).astype(bf16)
        cT = np.ascontiguousarray(cs[b].T).astype(bf16)
        in_maps.append({
            "hidT": hT,
            "crossT": cT,
            "hsliT": np.ascontiguousarray(hT[:, tsl]),
            "hsli": np.ascontiguousarray(hs[b, tsl, :]),
            "wq": np.ascontiguousarray(Wq[:, sl]).astype(bf16),
            "wk": np.ascontiguousarray(Wk[:, sl]).astype(bf16),
            "wv": np.ascontiguousarray(Wv[:, sl]).astype(bf16),
            "wo": np.ascontiguousarray(Wo[sl, :]).astype(bf16),
            "wg": Wg,
            "bq": np.ascontiguousarray(bq[sl].reshape(4, P, 1)),
            "bk": np.ascontiguousarray(bk[sl].reshape(4, P, 1)),
            "bvb": np.ascontiguousarray(np.broadcast_to(bv[sl], (P, QD))),
            "bob": bob,
            "bgb": bgb,
            "gmb": gmb,
            "btb": btb,
        })

    res = run_bass_kernel_spmd(
        nc, in_maps, core_ids=list(range(8)), trace=TRACE)
    LAST_RESULT = res

    out = np.empty((B, S, H), dtype=np.float32)
    for c in range(8):
        b, r = divmod(c, 4)
        out[b, r * TS:(r + 1) * TS, :] = res.results[c]["y"]
    return out



# revision 11
# speedup vs baseline: 1.3526x; 1.3526x over previous
"""CrossAttentionLayer Trainium2 kernel, 8-way sharded (v2).

Sharding: core c -> batch b = c//4, head-group/token-slice r = c%4.
- q/k/v projections column-sharded over heads (4 heads = 512 dims per core)
- attention per head in feature-major layout; plain bf16 exp (scores are
  ~1e-3 so softmax is near-uniform; bf16 noise on the attention path lands
  ~1e-5 relative on the final output, far under the 2e-2 gate)
- after each head's attnT [128, 2048] is done, a small AllToAll (512KB)
  exchanges token-slices across the 4 cores of the batch, so each core ends
  up with ALL 16 heads for ITS 512 tokens; the 4 A2As hide under attention
  compute of subsequent heads
- out-projection is then fully local (full 2048-dim contraction against a
  row-permuted full Wo); gate GEMM runs first as PE filler while the last
  A2A completes; LayerNorm per 128-token tile overlaps the out-proj
"""

import numpy as np

import concourse.bacc as bacc
import concourse.mybir as mybir
import concourse.tile as tile
from concourse.bass_utils import run_bass_kernel_spmd

H = 2048          # hidden
S = 2048          # sequence
B = 2             # batch
HD = 128          # head dim
P = 128           # partitions
QD = 512          # per-core qkv dims (4 heads)
TS = 512          # per-core token slice
KT = H // P       # 16 contraction tiles
ST = S // P       # 16 token tiles
SCALE = HD ** -0.5
EPS = 1e-5

F32 = mybir.dt.float32
BF16 = mybir.dt.bfloat16
FA = mybir.ActivationFunctionType
OP = mybir.AluOpType

TRACE = False          # test.py sets True to capture an NTFF profile
LAST_RESULT = None     # BassKernelResults from the most recent run

_CACHE = {}


def _build():
    from contextlib import ExitStack

    nc = bacc.Bacc("TRN2", target_bir_lowering=False, debug=False, num_devices=8)

    hidT = nc.dram_tensor("hidT", [H, S], BF16, kind="ExternalInput")
    crossT = nc.dram_tensor("crossT", [H, S], BF16, kind="ExternalInput")
    hsliT = nc.dram_tensor("hsliT", [H, TS], BF16, kind="ExternalInput")
    hsli = nc.dram_tensor("hsli", [TS, H], F32, kind="ExternalInput")
    wq = nc.dram_tensor("wq", [H, QD], BF16, kind="ExternalInput")
    wk = nc.dram_tensor("wk", [H, QD], BF16, kind="ExternalInput")
    wv = nc.dram_tensor("wv", [H, QD], BF16, kind="ExternalInput")
    wof = nc.dram_tensor("wof", [H, H], BF16, kind="ExternalInput")
    wg = nc.dram_tensor("wg", [H, H], BF16, kind="ExternalInput")
    bq = nc.dram_tensor("bq", [4, P, 1], F32, kind="ExternalInput")
    bk = nc.dram_tensor("bk", [4, P, 1], F32, kind="ExternalInput")
    bvb = nc.dram_tensor("bvb", [P, QD], F32, kind="ExternalInput")
    bob = nc.dram_tensor("bob", [P, H], F32, kind="ExternalInput")
    bgb = nc.dram_tensor("bgb", [P, H], F32, kind="ExternalInput")
    gmb = nc.dram_tensor("gmb", [P, H], F32, kind="ExternalInput")
    btb = nc.dram_tensor("btb", [P, H], F32, kind="ExternalInput")
    sel0 = nc.dram_tensor("sel0", [P, 1], F32, kind="ExternalInput")
    sel1 = nc.dram_tensor("sel1", [P, 1], F32, kind="ExternalInput")
    y = nc.dram_tensor("y", [TS, H], F32, kind="ExternalOutput")

    # A2A must span all 8 cores (4-core groups unsupported); each core
    # duplicates its chunk for dest-token-slice j into rank j and rank j+4,
    # and the receiver picks the same-batch copy with sel0/sel1 masks.
    groups8 = [[0, 1, 2, 3, 4, 5, 6, 7]]

    with tile.TileContext(nc) as tc, ExitStack() as top:
        const = top.enter_context(tc.tile_pool(name="const", bufs=1))
        ones_sq = const.tile([P, P], BF16, name="ones_sq")
        nc.gpsimd.memset(ones_sq[:], 1.0)
        eps_t = const.tile([P, 1], F32, name="eps_t")
        nc.gpsimd.memset(eps_t[:], EPS)
        bq_t = [const.tile([P, 1], F32, name=f"bq{m}") for m in range(4)]
        bk_t = [const.tile([P, 1], F32, name=f"bk{m}") for m in range(4)]
        for m in range(4):
            nc.gpsimd.dma_start(bq_t[m][:], bq[m])
            nc.gpsimd.dma_start(bk_t[m][:], bk[m])
        bvb_sb = const.tile([P, QD], BF16, name="bvb_sb")
        nc.gpsimd.dma_start(bvb_sb[:], bvb[:])
        sel0_t = const.tile([P, 1], F32, name="sel0_t")
        nc.gpsimd.dma_start(sel0_t[:], sel0[:])
        sel1_t = const.tile([P, 1], F32, name="sel1_t")
        nc.gpsimd.dma_start(sel1_t[:], sel1[:])

        cc = top.enter_context(tc.tile_pool(name="cc", bufs=1, space="DRAM"))
        cc_in = [cc.tile([8 * P, TS], BF16, name=f"ccin{h}") for h in range(4)]
        cc_out = [cc.tile([8 * P, TS], BF16, name=f"ccout{h}") for h in range(4)]

        hidT_r = hidT.rearrange("(t p) s -> t p s", p=P)
        crossT_r = crossT.rearrange("(t p) s -> t p s", p=P)
        wq_r = wq.rearrange("(t p) d -> t p d", p=P)
        wk_r = wk.rearrange("(t p) d -> t p d", p=P)
        wv_r = wv.rearrange("(t p) d -> t p d", p=P)
        wof_r = wof.rearrange("(t p) d -> t p d", p=P)
        wg_r = wg.rearrange("(t p) d -> t p d", p=P)
        hsliT_r = hsliT.rearrange("(t p) s -> t p s", p=P)

        # pools that survive into the out-proj/LN phases
        wop = top.enter_context(tc.tile_pool(name="wop", bufs=1))
        wof_sb = [wop.tile([P, H], BF16, name=f"wo{k}") for k in range(KT)]
        cs_p = top.enter_context(tc.tile_pool(name="cs", bufs=1))
        cs_sb = [cs_p.tile([P, TS], BF16, name=f"cs{k}") for k in range(KT)]

        with ExitStack() as ab:
            # ---- persistent activations for phases A+B ----
            qkv = ab.enter_context(tc.tile_pool(name="qkv", bufs=1))
            q_sb = [qkv.tile([P, S], BF16, name=f"q{m}") for m in range(4)]
            k_sb = [qkv.tile([P, S], BF16, name=f"k{m}") for m in range(4)]
            v_sb = [qkv.tile([P, QD], BF16, name=f"v{t}") for t in range(ST)]

            # ---- phase A: q projection ----
            with ExitStack() as ph:
                wp = ph.enter_context(tc.tile_pool(name="wp", bufs=1))
                xp = ph.enter_context(tc.tile_pool(name="xp", bufs=4))
                wq_sb = [wp.tile([P, QD], BF16, name=f"wq{k}") for k in range(KT)]
                wk_sb = [wp.tile([P, QD], BF16, name=f"wk{k}") for k in range(KT)]
                wv_sb = [wp.tile([P, QD], BF16, name=f"wv{k}") for k in range(KT)]
                for k in range(KT):
                    nc.sync.dma_start(wq_sb[k][:], wq_r[k])
                for k in range(KT):
                    nc.sync.dma_start(wk_sb[k][:], wk_r[k])
                    nc.sync.dma_start(wv_sb[k][:], wv_r[k])
                with tc.tile_pool(name="psA", bufs=8, space="PSUM") as psA:
                  for c in range(4):
                    ps_q = [psA.tile([P, 512], F32, name="psq") for _ in range(4)]
                    for k in range(KT):
                        x = xp.tile([P, 512], BF16, name="x")
                        nc.sync.dma_start(x[:], hidT_r[k, :, c * 512:(c + 1) * 512])
                        for m in range(4):
                            nc.tensor.matmul(
                                ps_q[m][:], wq_sb[k][:, m * P:(m + 1) * P], x[:],
                                start=(k == 0), stop=(k == KT - 1))
                    for m in range(4):
                        nc.scalar.activation(
                            q_sb[m][:, c * 512:(c + 1) * 512], ps_q[m][:],
                            FA.Identity, bias=bq_t[m][:])

                # ---- phase A: k and v projections (one crossT pass) ----
                with tc.tile_pool(name="psA2", bufs=4, space="PSUM") as psA2:
                  for c in range(4):
                    ps_k = [psA2.tile([P, 512], F32, name="psk") for _ in range(4)]
                    ps_v = [psA2.tile([P, 512], F32, name="psv") for _ in range(4)]
                    for k in range(KT):
                        x = xp.tile([P, 512], BF16, name="x2")
                        nc.sync.dma_start(x[:], crossT_r[k, :, c * 512:(c + 1) * 512])
                        for m in range(4):
                            nc.tensor.matmul(
                                ps_k[m][:], wk_sb[k][:, m * P:(m + 1) * P], x[:],
                                start=(k == 0), stop=(k == KT - 1))
                        for t in range(4):
                            nc.tensor.matmul(
                                ps_v[t][:], x[:, t * P:(t + 1) * P], wv_sb[k][:],
                                start=(k == 0), stop=(k == KT - 1))
                    for m in range(4):
                        nc.scalar.activation(
                            k_sb[m][:, c * 512:(c + 1) * 512], ps_k[m][:],
                            FA.Identity, bias=bk_t[m][:])
                    for t in range(4):
                        nc.vector.tensor_add(
                            v_sb[c * 4 + t][:], ps_v[t][:], bvb_sb[:])

            # ---- phase B: attention per head + per-head AllToAll ----
            # (woF prefetch rides the gpsimd queue ahead of the collectives)
            for k in range(KT):
                nc.gpsimd.dma_start(wof_sb[k][:], wof_r[k])
            with ExitStack() as ph:
                psS = ph.enter_context(tc.tile_pool(name="psS", bufs=4, space="PSUM"))
                psAcc = ph.enter_context(tc.tile_pool(name="psAcc", bufs=2, space="PSUM"))
                exp_p = ph.enter_context(tc.tile_pool(name="exp", bufs=8))
                tmp_p = ph.enter_context(tc.tile_pool(name="tmpB", bufs=4))
                attn_p = ph.enter_context(tc.tile_pool(name="attn", bufs=2))
                for h in range(4):
                    attnT = attn_p.tile([P, S], BF16, name="attnT")
                    for c in range(4):
                        ps_at = psAcc.tile([P, 512], F32, name="psat")
                        ps_sum = psAcc.tile([P, 512], F32, name="pssum")
                        for t in range(ST):
                            ps_sc = psS.tile([P, 512], F32, name="pssc")
                            nc.tensor.matmul(
                                ps_sc[:], k_sb[h][:, t * P:(t + 1) * P],
                                q_sb[h][:, c * 512:(c + 1) * 512],
                                start=True, stop=True)
                            ex = exp_p.tile([P, 512], BF16, name="ex")
                            nc.scalar.activation(ex[:], ps_sc[:], FA.Exp, scale=SCALE)
                            nc.tensor.matmul(
                                ps_at[:], v_sb[t][:, h * P:(h + 1) * P], ex[:],
                                start=(t == 0), stop=(t == ST - 1))
                            nc.tensor.matmul(
                                ps_sum[:], ones_sq[:], ex[:],
                                start=(t == 0), stop=(t == ST - 1))
                        rec = tmp_p.tile([P, 512], F32, name="rec")
                        nc.vector.reciprocal(rec[:], ps_sum[:])
                        nc.vector.tensor_mul(
                            attnT[:, c * 512:(c + 1) * 512], ps_at[:], rec[:])
                    for p in range(4):
                        nc.sync.dma_start(
                            cc_in[h][p * P:(p + 1) * P, :],
                            attnT[:, p * TS:(p + 1) * TS])
                        nc.sync.dma_start(
                            cc_in[h][(4 + p) * P:(5 + p) * P, :],
                            attnT[:, p * TS:(p + 1) * TS])
                    nc.gpsimd.collective_compute(
                        "AllToAll", OP.bypass, replica_groups=groups8,
                        ins=[cc_in[h][:].opt()], outs=[cc_out[h][:].opt()])
                    for p in range(4):
                        ca = tmp_p.tile([P, TS], BF16, name="ca")
                        cb = tmp_p.tile([P, TS], BF16, name="cb")
                        nc.gpsimd.dma_start(ca[:], cc_out[h][p * P:(p + 1) * P, :])
                        nc.gpsimd.dma_start(
                            cb[:], cc_out[h][(4 + p) * P:(5 + p) * P, :])
                        nc.vector.tensor_scalar_mul(ca[:], ca[:], sel0_t[:])
                        nc.vector.tensor_scalar_mul(cb[:], cb[:], sel1_t[:])
                        nc.vector.tensor_add(cs_sb[h * 4 + p][:], ca[:], cb[:])

        # ---- phases C..E share the gate output ----
        cde = top.enter_context(tc.tile_pool(name="gp", bufs=1))
        g_sb = [cde.tile([P, H], BF16, name=f"g{m}") for m in range(4)]
        bg_sb = cde.tile([P, H], BF16, name="bg_sb")
        nc.gpsimd.dma_start(bg_sb[:], bgb[:])
        bo_sb = cde.tile([P, H], BF16, name="bo_sb")
        nc.gpsimd.dma_start(bo_sb[:], bob[:])
        gm_sb = cde.tile([P, H], F32, name="gm_sb")
        nc.gpsimd.dma_start(gm_sb[:], gmb[:])
        bt_sb = cde.tile([P, H], F32, name="bt_sb")
        nc.gpsimd.dma_start(bt_sb[:], btb[:])

        # ---- phase C: gate GEMM (PE filler while the last A2A lands) ----
        with ExitStack() as ph:
            hsl_p = ph.enter_context(tc.tile_pool(name="hsl", bufs=1))
            wgp = ph.enter_context(tc.tile_pool(name="wgp", bufs=6))
            psG = ph.enter_context(tc.tile_pool(name="psG", bufs=4, space="PSUM"))
            fin0 = ph.enter_context(tc.tile_pool(name="fin0", bufs=2))
            hsl_sb = [hsl_p.tile([P, 512], BF16, name=f"hs{k}") for k in range(KT)]
            for k in range(KT):
                nc.sync.dma_start(hsl_sb[k][:], hsliT_r[k])
            for n in range(4):
                ps_g = [psG.tile([P, 512], F32, name="psg") for _ in range(4)]
                for k in range(KT):
                    wgt = wgp.tile([P, 512], BF16, name="wgt")
                    nc.sync.dma_start(wgt[:], wg_r[k, :, n * 512:(n + 1) * 512])
                    for m in range(4):
                        nc.tensor.matmul(
                            ps_g[m][:], hsl_sb[k][:, m * P:(m + 1) * P], wgt[:],
                            start=(k == 0), stop=(k == KT - 1))
                for m in range(4):
                    t = fin0.tile([P, 512], F32, name="gpre")
                    nc.vector.tensor_add(
                        t[:], ps_g[m][:], bg_sb[:, n * 512:(n + 1) * 512])
                    nc.scalar.activation(
                        g_sb[m][:, n * 512:(n + 1) * 512], t[:], FA.Sigmoid)

        # ---- phase D+E: local out-projection + LayerNorm per token tile ----
        with ExitStack() as ph:
            psD = ph.enter_context(tc.tile_pool(name="psD", bufs=8, space="PSUM"))
            fin = ph.enter_context(tc.tile_pool(name="fin", bufs=2))
            res_p = ph.enter_context(tc.tile_pool(name="res", bufs=2))
            sml = ph.enter_context(tc.tile_pool(name="sml", bufs=4))
            for m in range(4):
                x_res = res_p.tile([P, H], F32, name="xres")
                nc.sync.dma_start(x_res[:], hsli[m * P:(m + 1) * P, :])
                ps_o = [psD.tile([P, 512], F32, name="pso") for _ in range(4)]
                for k in range(KT):
                    for n in range(4):
                        nc.tensor.matmul(
                            ps_o[n][:], cs_sb[k][:, m * P:(m + 1) * P],
                            wof_sb[k][:, n * 512:(n + 1) * 512],
                            start=(k == 0), stop=(k == KT - 1))
                o = fin.tile([P, H], F32, name="o")
                for n in range(4):
                    nc.vector.tensor_add(
                        o[:, n * 512:(n + 1) * 512], ps_o[n][:],
                        bo_sb[:, n * 512:(n + 1) * 512])
                nc.vector.tensor_mul(o[:], o[:], g_sb[m][:])
                nc.vector.tensor_add(o[:], o[:], x_res[:])
                st6 = sml.tile([P, 4, 6], F32, name="st6")
                for cch in range(4):
                    nc.vector.bn_stats(
                        st6[:, cch, :], o[:, cch * 512:(cch + 1) * 512])
                mv = sml.tile([P, 2], F32, name="mv")
                nc.vector.bn_aggr(mv[:], st6[:])
                nmean = sml.tile([P, 1], F32, name="nmean")
                nc.scalar.mul(nmean[:], mv[:, 0:1], -1.0)
                sd = sml.tile([P, 1], F32, name="sd")
                nc.scalar.activation(sd[:], mv[:, 1:2], FA.Sqrt, bias=eps_t[:], scale=1.0)
                rstd = sml.tile([P, 1], F32, name="rstd")
                nc.vector.reciprocal(rstd[:], sd[:])
                nc.vector.tensor_scalar(
                    o[:], o[:], nmean[:], rstd[:],
                    op0=OP.add, op1=OP.mult)
                nc.vector.tensor_mul(o[:], o[:], gm_sb[:])
                nc.vector.tensor_add(o[:], o[:], bt_sb[:])
                nc.sync.dma_start(y[m * P:(m + 1) * P, :], o[:])

    nc.compile()
    return nc


def kernel(**inputs):
    global LAST_RESULT
    import ml_dtypes

    if "nc" not in _CACHE:
        _CACHE["nc"] = _build()
    nc = _CACHE["nc"]

    bf16 = ml_dtypes.bfloat16
    hs = np.asarray(inputs["hidden_states"], dtype=np.float32)
    cs = np.asarray(inputs["cross_states"], dtype=np.float32)
    Wq = np.asarray(inputs["Wq"], dtype=np.float32)
    Wk = np.asarray(inputs["Wk"], dtype=np.float32)
    Wv = np.asarray(inputs["Wv"], dtype=np.float32)
    Wo = np.asarray(inputs["Wo"], dtype=np.float32)
    Wg = np.asarray(inputs["Wg"], dtype=np.float32).astype(bf16)
    bq = np.asarray(inputs["bq"], dtype=np.float32)
    bk = np.asarray(inputs["bk"], dtype=np.float32)
    bv = np.asarray(inputs["bv"], dtype=np.float32)
    bo = np.asarray(inputs["bo"], dtype=np.float32)
    bg = np.asarray(inputs["bg"], dtype=np.float32)
    gm = np.asarray(inputs["ln_gamma"], dtype=np.float32)
    bt = np.asarray(inputs["ln_beta"], dtype=np.float32)

    bob = np.ascontiguousarray(np.broadcast_to(bo, (P, H)))
    bgb = np.ascontiguousarray(np.broadcast_to(bg, (P, H)))
    gmb = np.ascontiguousarray(np.broadcast_to(gm, (P, H)))
    btb = np.ascontiguousarray(np.broadcast_to(bt, (P, H)))

    # Wo with rows permuted to the A2A delivery order: block (h, p) holds
    # global head 4p+h (cs_sb[h*4+p] carries head 4p+h of the core's tokens)
    perm = [4 * p + h for h in range(4) for p in range(4)]
    woF = np.concatenate([Wo[g * HD:(g + 1) * HD, :] for g in perm], axis=0)
    woF = np.ascontiguousarray(woF).astype(bf16)

    in_maps = []
    for c in range(8):
        b, r = divmod(c, 4)
        sl = slice(r * QD, (r + 1) * QD)
        tsl = slice(r * TS, (r + 1) * TS)
        hT = np.ascontiguousarray(hs[b].T).astype(bf16)
        cT = np.ascontiguousarray(cs[b].T).astype(bf16)
        in_maps.append({
            "hidT": hT,
            "crossT": cT,
            "hsliT": np.ascontiguousarray(hT[:, tsl]),
            "hsli": np.ascontiguousarray(hs[b, tsl, :]),
            "wq": np.ascontiguousarray(Wq[:, sl]).astype(bf16),
            "wk": np.ascontiguousarray(Wk[:, sl]).astype(bf16),
            "wv": np.ascontiguousarray(Wv[:, sl]).astype(bf16),
            "wof": woF,
            "wg": Wg,
            "bq": np.ascontiguousarray(bq[sl].reshape(4, P, 1)),
            "bk": np.ascontiguousarray(bk[sl].reshape(4, P, 1)),
            "bvb": np.ascontiguousarray(np.broadcast_to(bv[sl], (P, QD))),
            "bob": bob,
            "bgb": bgb,
            "gmb": gmb,
            "btb": btb,
            "sel0": np.full((P, 1), 1.0 if b == 0 else 0.0, np.float32),
            "sel1": np.full((P, 1), 0.0 if b == 0 else 1.0, np.float32),
        })

    res = run_bass_kernel_spmd(
        nc, in_maps, core_ids=list(range(8)), trace=TRACE)
    LAST_RESULT = res

    out = np.empty((B, S, H), dtype=np.float32)
    for c in range(8):
        b, r = divmod(c, 4)
        out[b, r * TS:(r + 1) * TS, :] = res.results[c]["y"]
    return out
